# revision 19
# baseline (speedup 1.0000x reference)
"""MoE BaseLayer kernel for Trainium2 (8 NeuronCores, expert parallelism).

Strategy (per the expert-parallelism sharding hint):
  * Host computes token->expert assignment (scores = x @ centroids.T, argmax)
    -- this IS the shard function: tokens are dispatched to the core owning
    their expert (the host-side equivalent of the All2All in the original),
    and the gate alpha = sigmoid(score of the assigned expert) falls out of
    the same routing scores. The host also applies the per-token LayerNorm
    and ships the normalized activations pre-transposed (D-major), so the
    device runs no LN chain and no PE transposes at all.
  * Core e holds expert e's weights only and runs the expert FFN
    (FF1 -> ReLU -> FF2) + alpha blend for its routed tokens. LayerNorm's
    affine (ln_g, ln_b) is folded into W1/b1, and alpha*b2 is folded into
    the residual tile (y = x + a*(ff+b2) = (x + a*b2) + a*ff), both exact
    reparameterizations.
  * Host scatters per-core outputs back to original token order (combine).

Device kernel (per core, C padded routed tokens), v4 tuned from traces:
  * weights cast to bf16 on the host: halves the 8MB/core weight stream
    and enables the PE's automatic Fast Weight Load (fp32-disabled).
    End-to-end absmax rel err ~2e-3 vs the 2e-2 gate.
  * DMA: the gpsimd (SWDGE) queue starts ~3us before the sync (HWDGE)
    queue, so the critical head of the stream (meta, xlnT slab, w1 quad 0)
    rides gpsimd; the rest (w2q0, w1g1..w2q3, then the residual xs tiles,
    needed only at the tail) streams on the sync FIFO in consumption order.
  * PE: a short warm-up spin keeps the PE continuously busy from kernel
    start -- the HAM clock governor grants 2.4GHz only after ~7.5us of
    sustained PE activity, so the spin starts that clock immediately and
    hands off to FF1 with no gap.
  * FF1 (w1 stationary, H^T F-major) with ReLU+bias on ACT -> bf16; FF2
    (h stationary, w2 moving) software-pipelined one F-tile behind FF1.
  * blend y = xs2 + alpha*yacc via ACT scale-copy + DVE residual add.
"""

import numpy as np
import ml_dtypes

E, D, F = 8, 512, 2048
LN_EPS = 1e-5
P = 128

_CACHE = {}


def _build(C):
    import concourse.tile as tile
    from concourse import bacc, mybir

    f32 = mybir.dt.float32
    bf16 = mybir.dt.bfloat16
    ACT = mybir.ActivationFunctionType
    NT = -(-C // P)       # token tiles (last may be partial, C % 64 == 0)
    SZ = [min(P, C - i * P) for i in range(NT)]   # rows per token tile
    KT = D // P           # contraction tiles over D (4)
    FT = F // P           # F tiles (16)
    NG = (NT + 3) // 4    # groups of <=512 tokens (PSUM bank limit)

    nc = bacc.Bacc("TRN2", target_bir_lowering=False, num_devices=E)
    xlnT_d = nc.dram_tensor("xlnT", [P, KT * C], bf16, kind="ExternalInput")
    xs_d = nc.dram_tensor("xs", [NT, P, D], f32, kind="ExternalInput")
    meta_d = nc.dram_tensor("meta", [P, FT + NT], f32, kind="ExternalInput")
    wall_d = nc.dram_tensor("wall", [2 * (FT // 4), P, KT * 512], bf16,
                            kind="ExternalInput")
    y_d = nc.dram_tensor("y", [C, D], f32, kind="ExternalOutput")
    scr_d = nc.dram_tensor("scr", [P, 1], f32, kind="ExternalOutput")

    with tile.TileContext(nc) as tc:
        with (
            tc.tile_pool(name="consts", bufs=1) as consts,
            tc.tile_pool(name="wpool", bufs=1) as wpool,
            tc.tile_pool(name="xpool", bufs=1) as xpool,
            tc.tile_pool(name="hpool", bufs=5) as hpool,
            tc.tile_pool(name="opool", bufs=3) as opool,
            tc.tile_pool(name="pf1", bufs=2, space="PSUM") as pf1,
            tc.tile_pool(name="pf2", bufs=1, space="PSUM") as pf2,
            tc.tile_pool(name="pwarm", bufs=1, space="PSUM") as pwarm,
        ):
            w1g = [None] * (FT // 4)
            w2q = [None] * (FT // 4)

            def load_w1g(g, eng):
                t = wpool.tile([P, KT, 512], bf16, name=f"w1g{g}", tag=f"w1g{g}")
                eng.dma_start(
                    out=t,
                    in_=wall_d[2 * g].rearrange("p (k f) -> p k f", k=KT),
                )
                w1g[g] = t

            def load_w2q(g, eng):
                t = wpool.tile([P, 4, D], bf16, name=f"w2q{g}", tag=f"w2q{g}")
                eng.dma_start(
                    out=t,
                    in_=wall_d[2 * g + 1].rearrange("p (q d) -> p q d", q=4),
                )
                w2q[g] = t

            # ---- tiny meta rides gpsimd (SWDGE fires early, parallel to
            # the sync FIFO); all bulk data streams on the sync (HWDGE)
            # queue in consumption order -- SWDGE's data path is ~3x slower
            # for large transfers, so only metadata goes there
            meta_t = xpool.tile([P, FT + NT], f32, name="meta_t", tag="meta_t")
            nc.gpsimd.dma_start(out=meta_t, in_=meta_d[:])
            xlnT_t = xpool.tile([P, KT * C], bf16, name="xlnT", tag="xlnT")
            nc.sync.dma_start(out=xlnT_t, in_=xlnT_d[:])
            b1T = meta_t[:, :FT]
            alT = [meta_t[: SZ[i], FT + i:FT + i + 1] for i in range(NT)]

            # quad 0 streams at single-F-tile granularity so FF1 starts the
            # moment the first w1 tile lands (~2us earlier than quad-sized
            # transfers), interleaved with the first w2 tiles in exact
            # consumption order
            w1f = [None] * 4
            # wall block 0 is packed F-tile-major on the host: each ft's
            # [P, KT*128] sub-block is per-partition contiguous, so these
            # single-F-tile transfers keep 1KB descriptors
            wall0 = wall_d[0].rearrange("p (f k x) -> p f k x", f=4, k=KT)
            wall1 = wall_d[1].rearrange("p (q d) -> p q d", q=4)

            def load_w1f(ft):
                t = wpool.tile([P, KT, P], bf16, name=f"w1f{ft}", tag=f"w1f{ft}")
                nc.sync.dma_start(out=t, in_=wall0[:, ft])
                w1f[ft] = t

            load_w1f(0)
            load_w1f(1)
            w2t0 = wpool.tile([P, D], bf16, name="w2t0", tag="w2t0")
            nc.sync.dma_start(out=w2t0, in_=wall1[:, 0, :])
            load_w1f(2)
            load_w1f(3)
            w2t1 = wpool.tile([P, D], bf16, name="w2t1", tag="w2t1")
            nc.sync.dma_start(out=w2t1, in_=wall1[:, 1, :])
            w2r0 = wpool.tile([P, 2, D], bf16, name="w2r0", tag="w2r0")
            nc.sync.dma_start(out=w2r0, in_=wall1[:, 2:4, :])
            for g in range(1, FT // 4):
                load_w1g(g, nc.sync)
                load_w2q(g, nc.sync)

            # residual xs2 (= xs + alpha*b2) only feeds the tail blend
            xs_t = []
            for i in range(NT):
                t = xpool.tile([P, D], f32, name=f"xs{i}", tag=f"xs{i}")
                nc.sync.dma_start(out=t, in_=xs_d[i])
                xs_t.append(t)

            # ---- warm-up spin: PE continuously busy from kernel start so
            # the HAM governor's 2.4GHz grant (~7.5us of sustained PE
            # activity) arrives as early as possible
            warmA = consts.tile([P, P], bf16, name="warmA", tag="warmA")
            nc.vector.memset(warmA, 0.0)
            warmB = consts.tile([P, 512], bf16, name="warmB", tag="warmB")
            nc.vector.memset(warmB, 0.0)
            wkeep = consts.tile([P, 1], f32, name="wkeep", tag="wkeep")
            wps = pwarm.tile([P, 512], f32, name="wps", tag="wps")
            N_WARM = 7
            for wi in range(N_WARM):
                nc.tensor.matmul(
                    wps, warmA, warmB, start=(wi == 0), stop=(wi == N_WARM - 1)
                )

            # ---- per-group compute ----------------------------------------
            for grp in range(NG):
                t0 = grp * 4                      # first token tile of group
                tn = min(4, NT - t0)              # tiles in this group
                Cg = sum(SZ[t0:t0 + tn])
                cols = [sum(SZ[t0:i]) for i in range(t0, t0 + tn)]

                def xlnT_ap(kt):
                    return xlnT_t[:, kt * C + t0 * P: kt * C + t0 * P + Cg]

                yaccs = [
                    pf2.tile([P, D], f32, name=f"yacc{i - t0}", tag=f"yacc{i - t0}")
                    for i in range(t0, t0 + tn)
                ]

                # FF1 + FF2, software-pipelined one F-tile apart
                hs = [None] * FT

                def w1_ap(ft, kt):
                    if grp == 0 and ft < 4:
                        return w1f[ft][:, kt, :]
                    return w1g[ft // 4][:, kt, (ft % 4) * P:(ft % 4 + 1) * P]

                def w2_ap(ft):
                    if grp == 0:
                        if ft == 0:
                            return w2t0
                        if ft == 1:
                            return w2t1
                        if ft < 4:
                            return w2r0[:, ft - 2, :]
                    return w2q[ft // 4][:, ft % 4, :]

                def ff1(ft):
                    acc = pf1.tile([P, Cg], f32, name="acc1", tag="acc1")
                    for kt in range(KT):
                        nc.tensor.matmul(
                            acc, w1_ap(ft, kt), xlnT_ap(kt),
                            start=(kt == 0), stop=(kt == KT - 1),
                        )
                    h = hpool.tile([P, Cg], bf16, name="h", tag="h")
                    nc.scalar.activation(
                        out=h, in_=acc, func=ACT.Relu,
                        bias=b1T[:, ft:ft + 1], scale=1.0,
                    )
                    hs[ft] = h

                def ff2(ft, i):
                    col = cols[i - t0]
                    nc.tensor.matmul(
                        yaccs[i - t0][: SZ[i]],
                        hs[ft][:, col:col + SZ[i]],
                        w2_ap(ft),
                        start=(ft == 0), stop=(ft == FT - 1),
                    )

                # software pipeline: ff2(ft-1) rides one F-tile behind
                # ff1(ft); the last quad's ff2 runs tile-major instead so
                # each token tile's accumulator closes early and its blend +
                # output DMA overlap the remaining matmuls
                ff1(0)
                for ft in range(1, FT):
                    ff1(ft)
                    if ft - 1 < FT - 4:
                        for i in range(t0, t0 + tn):
                            ff2(ft - 1, i)
                for i in range(t0, t0 + tn):
                    sz = SZ[i]
                    for ft in range(FT - 4, FT):
                        ff2(ft, i)
                    # blend: y = xs2 + alpha*yacc (alpha*b2 inside xs2)
                    yo = opool.tile([P, D], f32, name="yo", tag="yo")
                    nc.scalar.activation(
                        out=yo[:sz], in_=yaccs[i - t0][:sz],
                        func=ACT.Copy, scale=alT[i],
                    )
                    nc.vector.tensor_add(
                        out=yo[:sz], in0=yo[:sz], in1=xs_t[i][:sz]
                    )
                    nc.sync.dma_start(
                        out=y_d[i * P:i * P + sz, :], in_=yo[:sz]
                    )

            # keep-alive so DCE cannot drop the warm-up chain; rides the
            # gpsimd queue at the very end so it never stalls weight DMAs
            nc.scalar.activation(out=wkeep, in_=wps[:, 0:1], func=ACT.Copy)
            nc.gpsimd.dma_start(out=scr_d[:], in_=wkeep)

    nc.compile()
    return nc


def _get_nc(C):
    if C not in _CACHE:
        _CACHE[C] = _build(C)
    return _CACHE[C]


def _route(feats, centroids):
    """Token->expert assignment + gate, computed the same way the reference
    does (jax on CPU) so argmax near-ties resolve identically."""
    try:
        import jax
        import jax.numpy as jnp

        with jax.default_device(jax.devices("cpu")[0]):
            scores = jnp.asarray(feats) @ jnp.asarray(centroids).T
            assign = jnp.argmax(scores, axis=1)
            alpha = jax.nn.sigmoid(
                jnp.take_along_axis(scores, assign[:, None], axis=1)
            )
            return np.asarray(assign), np.asarray(alpha, dtype=np.float32)
    except Exception:
        scores = feats @ centroids.T
        assign = np.argmax(scores, axis=1)
        alpha = 1.0 / (1.0 + np.exp(-scores[np.arange(len(assign)), assign]))
        return assign, alpha[:, None].astype(np.float32)


def prepare(x, centroids, ln_g, ln_b, W1, b1, W2, b2):
    """Shard the full inputs: route tokens to experts, apply LN, build
    per-core input maps. Returns (C, in_maps, idx, orig_shape)."""
    bf16 = ml_dtypes.bfloat16
    x = np.asarray(x)
    orig_shape = x.shape
    feats = np.ascontiguousarray(x.reshape(-1, D), dtype=np.float32)
    centroids = np.asarray(centroids, dtype=np.float32)

    assign, alpha = _route(feats, centroids)

    idx = [np.nonzero(assign == e)[0] for e in range(E)]
    max_count = max(len(ix) for ix in idx)
    C = max(256, -(-max_count // 64) * 64)

    W1 = np.asarray(W1, dtype=np.float32)
    W2 = np.asarray(W2, dtype=np.float32)
    b1 = np.asarray(b1, dtype=np.float32)
    b2 = np.asarray(b2, dtype=np.float32)
    ln_g = np.asarray(ln_g, dtype=np.float32)
    ln_b = np.asarray(ln_b, dtype=np.float32)

    # per-token LN on the host (the device receives normalized, transposed
    # activations)
    mu = feats.mean(axis=1, keepdims=True)
    var = np.square(feats - mu).mean(axis=1)
    rs = (1.0 / np.sqrt(var + LN_EPS))[:, None]
    xnorm = (feats - mu) * rs

    NT = -(-C // P)
    FT = F // P
    KT = D // P
    in_maps = []
    for e in range(E):
        ne = len(idx[e])
        al = np.zeros((NT * P,), dtype=np.float32)
        al[:ne] = alpha[idx[e], 0]
        # residual with alpha*b2 folded in (exact reparameterization)
        xs2 = np.zeros((NT * P, D), dtype=np.float32)
        xs2[:ne] = feats[idx[e]] + al[:ne, None] * b2[e][None, :]
        # normalized activations, bf16, transposed to D-major [P, KT*C]
        xp = np.zeros((NT * P, D), dtype=np.float32)
        xp[:ne] = xnorm[idx[e]]
        xp = xp[:C].astype(bf16)
        xlnT = np.concatenate(
            [xp[:, k * P:(k + 1) * P].T for k in range(KT)], axis=1
        )
        # fold LN affine into the first FFN layer (exact reparameterization)
        w1_eff = ln_g[e][:, None] * W1[e]
        b1_eff = ln_b[e] @ W1[e] + b1[e]

        meta = np.empty((P, FT + NT), dtype=np.float32)
        meta[:, :FT] = b1_eff.reshape(FT, P).T
        meta[:, FT:] = al.reshape(NT, P).T

        wall = np.empty((2 * (FT // 4), P, KT * 512), dtype=bf16)
        for g in range(FT // 4):
            if g == 0:
                # quad 0 is F-tile-major: per ft a contiguous [P, KT*128]
                # sub-block (kt-major inside), enabling efficient
                # single-F-tile DMA slices for the stream head
                wall[0] = np.concatenate(
                    [
                        w1_eff[:, ft * P:(ft + 1) * P]
                        .reshape(KT, P, P).transpose(1, 0, 2).reshape(P, KT * P)
                        for ft in range(4)
                    ],
                    axis=1,
                ).astype(bf16)
            else:
                wall[2 * g] = (
                    w1_eff[:, g * 512:(g + 1) * 512]
                    .reshape(KT, P, 512).transpose(1, 0, 2).reshape(P, KT * 512)
                    .astype(bf16)
                )
            wall[2 * g + 1] = (
                W2[e][4 * g * P:(4 * g + 4) * P, :]
                .reshape(4, P, D).transpose(1, 0, 2).reshape(P, 4 * D)
                .astype(bf16)
            )
        in_maps.append(
            dict(
                xlnT=np.ascontiguousarray(xlnT),
                xs=np.ascontiguousarray(xs2.reshape(NT, P, D)),
                meta=meta,
                wall=wall,
            )
        )
    return C, in_maps, idx, orig_shape


def kernel(x, centroids, ln_g, ln_b, W1, b1, W2, b2):
    from concourse.bass_utils import run_bass_kernel_spmd

    C, in_maps, idx, orig_shape = prepare(
        x, centroids, ln_g, ln_b, W1, b1, W2, b2
    )
    nc = _get_nc(C)
    res = run_bass_kernel_spmd(nc, in_maps, core_ids=list(range(E)))

    T = int(np.prod(orig_shape[:-1]))
    out = np.empty((T, D), dtype=np.float32)
    for e in range(E):
        out[idx[e]] = res.results[e]["y"][: len(idx[e])]
    return out.reshape(orig_shape)


# revision 25
# speedup vs baseline: 1.2528x; 1.2528x over previous
"""MoE BaseLayer kernel for Trainium2 (8 NeuronCores, expert parallelism).

Strategy (per the expert-parallelism sharding hint):
  * Host computes token->expert assignment (scores = x @ centroids.T, argmax)
    -- this IS the shard function: tokens are dispatched to the core owning
    their expert (the host-side equivalent of the All2All in the original),
    and the gate alpha = sigmoid(score of the assigned expert) falls out of
    the same routing scores. The host also applies the per-token LayerNorm
    and ships the normalized activations pre-transposed (D-major), so the
    device runs no LN chain and no PE transposes at all.
  * Core e holds expert e's weights only and runs the expert FFN
    (FF1 -> ReLU -> FF2) + alpha blend for its routed tokens. LayerNorm's
    affine (ln_g, ln_b) is folded into W1/b1, and alpha*b2 is folded into
    the residual tile (y = x + a*(ff+b2) = (x + a*b2) + a*ff), both exact
    reparameterizations.
  * Host scatters per-core outputs back to original token order (combine).

Device kernel (per core, C padded routed tokens), v4 tuned from traces:
  * weights cast to bf16 on the host: halves the 8MB/core weight stream
    and enables the PE's automatic Fast Weight Load (fp32-disabled).
    End-to-end absmax rel err ~2e-3 vs the 2e-2 gate.
  * DMA: the gpsimd (SWDGE) queue starts ~3us before the sync (HWDGE)
    queue, so the critical head of the stream (meta, xlnT slab, w1 quad 0)
    rides gpsimd; the rest (w2q0, w1g1..w2q3, then the residual xs tiles,
    needed only at the tail) streams on the sync FIFO in consumption order.
  * PE: a short warm-up spin keeps the PE continuously busy from kernel
    start -- the HAM clock governor grants 2.4GHz only after ~7.5us of
    sustained PE activity, so the spin starts that clock immediately and
    hands off to FF1 with no gap.
  * FF1 (w1 stationary, H^T F-major) with ReLU+bias on ACT -> bf16; FF2
    (h stationary, w2 moving) software-pipelined one F-tile behind FF1.
  * blend y = xs2 + alpha*yacc via ACT scale-copy + DVE residual add.
"""

import numpy as np
import ml_dtypes

E, D, F = 8, 512, 2048
LN_EPS = 1e-5
P = 128

_CACHE = {}


def _build(C):
    import concourse.tile as tile
    from concourse import bacc, mybir

    f32 = mybir.dt.float32
    bf16 = mybir.dt.bfloat16
    ACT = mybir.ActivationFunctionType
    NT = -(-C // P)       # token tiles (last may be partial, C % 64 == 0)
    SZ = [min(P, C - i * P) for i in range(NT)]   # rows per token tile
    KT = D // P           # contraction tiles over D (4)
    FT = F // P           # F tiles (16)
    NG = (NT + 3) // 4    # groups of <=512 tokens (PSUM bank limit)

    nc = bacc.Bacc("TRN2", target_bir_lowering=False, num_devices=E)
    # head = xlnT slab + w2 quad-0 tile-0, one transfer (dispatches are
    # ~700ns of serial sequencer time each -- minimize transfer count)
    xlnT_d = nc.dram_tensor("xlnT", [P, KT * C + D], bf16, kind="ExternalInput")
    xs_d = nc.dram_tensor("xs", [NT, P, D], f32, kind="ExternalInput")
    meta_d = nc.dram_tensor("meta", [P, FT + NT], f32, kind="ExternalInput")
    wall_d = nc.dram_tensor("wall", [2 * (FT // 4), P, KT * 512], bf16,
                            kind="ExternalInput")
    y_d = nc.dram_tensor("y", [C, D], f32, kind="ExternalOutput")
    scr_d = nc.dram_tensor("scr", [P, 1], f32, kind="ExternalOutput")

    with tile.TileContext(nc) as tc:
        with (
            tc.tile_pool(name="consts", bufs=1) as consts,
            tc.tile_pool(name="wpool", bufs=1) as wpool,
            tc.tile_pool(name="xpool", bufs=1) as xpool,
            tc.tile_pool(name="hpool", bufs=5) as hpool,
            tc.tile_pool(name="opool", bufs=3) as opool,
            tc.tile_pool(name="pf1", bufs=2, space="PSUM") as pf1,
            tc.tile_pool(name="pf2", bufs=1, space="PSUM") as pf2,
            tc.tile_pool(name="pwarm", bufs=1, space="PSUM") as pwarm,
        ):
            w1g = [None] * (FT // 4)
            w2q = [None] * (FT // 4)

            def load_w1g(g, eng):
                t = wpool.tile([P, KT, 512], bf16, name=f"w1g{g}", tag=f"w1g{g}")
                eng.dma_start(
                    out=t,
                    in_=wall_d[2 * g].rearrange("p (k f) -> p k f", k=KT),
                )
                w1g[g] = t

            def load_w2q(g, eng):
                t = wpool.tile([P, 4, D], bf16, name=f"w2q{g}", tag=f"w2q{g}")
                eng.dma_start(
                    out=t,
                    in_=wall_d[2 * g + 1].rearrange("p (q d) -> p q d", q=4),
                )
                w2q[g] = t

            # ---- tiny meta rides gpsimd (SWDGE fires early, parallel to
            # the sync FIFO); all bulk data streams on the sync (HWDGE)
            # queue in consumption order -- SWDGE's data path is ~3x slower
            # for large transfers, so only metadata goes there
            meta_t = xpool.tile([P, FT + NT], f32, name="meta_t", tag="meta_t")
            nc.gpsimd.dma_start(out=meta_t, in_=meta_d[:])
            xlnT_t = xpool.tile([P, KT * C + D], bf16, name="xlnT", tag="xlnT")
            nc.sync.dma_start(out=xlnT_t, in_=xlnT_d[:])
            b1T = meta_t[:, :FT]
            alT = [meta_t[: SZ[i], FT + i:FT + i + 1] for i in range(NT)]
            w2t0 = xlnT_t[:, KT * C:]        # w2 quad-0 tile-0, in the head

            load_w1g(0, nc.sync)
            # w2 quad 0 ships without its first tile (it's in the head)
            wall1 = wall_d[1].rearrange("p (q d) -> p q d", q=4)
            w2q0r = wpool.tile([P, 3, D], bf16, name="w2q0r", tag="w2q0r")
            nc.sync.dma_start(out=w2q0r, in_=wall1[:, 1:4, :])
            for g in range(1, FT // 4):
                load_w1g(g, nc.sync)
                load_w2q(g, nc.sync)

            # residual xs2 (= xs + alpha*b2) only feeds the tail blend
            xs_t = []
            for i in range(NT):
                t = xpool.tile([P, D], f32, name=f"xs{i}", tag=f"xs{i}")
                nc.sync.dma_start(out=t, in_=xs_d[i])
                xs_t.append(t)

            # ---- warm-up spin: PE continuously busy from kernel start so
            # the HAM governor's 2.4GHz grant (~7.5us of sustained PE
            # activity) arrives as early as possible
            warmA = consts.tile([P, P], bf16, name="warmA", tag="warmA")
            nc.vector.memset(warmA, 0.0)
            warmB = consts.tile([P, 512], bf16, name="warmB", tag="warmB")
            nc.vector.memset(warmB, 0.0)
            wkeep = consts.tile([P, 1], f32, name="wkeep", tag="wkeep")
            wps = pwarm.tile([P, 512], f32, name="wps", tag="wps")
            N_WARM = 8
            for wi in range(N_WARM):
                nc.tensor.matmul(
                    wps, warmA, warmB, start=(wi == 0), stop=(wi == N_WARM - 1)
                )

            # ---- per-group compute ----------------------------------------
            for grp in range(NG):
                t0 = grp * 4                      # first token tile of group
                tn = min(4, NT - t0)              # tiles in this group
                Cg = sum(SZ[t0:t0 + tn])
                cols = [sum(SZ[t0:i]) for i in range(t0, t0 + tn)]

                def xlnT_ap(kt):
                    return xlnT_t[:, kt * C + t0 * P: kt * C + t0 * P + Cg]

                yaccs = [
                    pf2.tile([P, D], f32, name=f"yacc{i - t0}", tag=f"yacc{i - t0}")
                    for i in range(t0, t0 + tn)
                ]

                # FF1 + FF2, software-pipelined one F-tile apart
                hs = [None] * FT

                def w1_ap(ft, kt):
                    return w1g[ft // 4][:, kt, (ft % 4) * P:(ft % 4 + 1) * P]

                def w2_ap(ft):
                    if ft == 0:
                        return w2t0
                    if ft < 4:
                        return w2q0r[:, ft - 1, :]
                    return w2q[ft // 4][:, ft % 4, :]

                def ff1(ft):
                    acc = pf1.tile([P, Cg], f32, name="acc1", tag="acc1")
                    for kt in range(KT):
                        nc.tensor.matmul(
                            acc, w1_ap(ft, kt), xlnT_ap(kt),
                            start=(kt == 0), stop=(kt == KT - 1),
                        )
                    h = hpool.tile([P, Cg], bf16, name="h", tag="h")
                    nc.scalar.activation(
                        out=h, in_=acc, func=ACT.Relu,
                        bias=b1T[:, ft:ft + 1], scale=1.0,
                    )
                    hs[ft] = h

                def ff2(ft, i):
                    col = cols[i - t0]
                    nc.tensor.matmul(
                        yaccs[i - t0][: SZ[i]],
                        hs[ft][:, col:col + SZ[i]],
                        w2_ap(ft),
                        start=(ft == 0), stop=(ft == FT - 1),
                    )

                # software pipeline: ff2(ft-1) rides one F-tile behind
                # ff1(ft); the last quad's ff2 runs tile-major instead so
                # each token tile's accumulator closes early and its blend +
                # output DMA overlap the remaining matmuls
                ff1(0)
                for ft in range(1, FT):
                    ff1(ft)
                    if ft - 1 < FT - 4:
                        for i in range(t0, t0 + tn):
                            ff2(ft - 1, i)
                for i in range(t0, t0 + tn):
                    sz = SZ[i]
                    for ft in range(FT - 4, FT):
                        ff2(ft, i)
                    # blend: y = xs2 + alpha*yacc (alpha*b2 inside xs2)
                    yo = opool.tile([P, D], f32, name="yo", tag="yo")
                    nc.scalar.activation(
                        out=yo[:sz], in_=yaccs[i - t0][:sz],
                        func=ACT.Copy, scale=alT[i],
                    )
                    nc.vector.tensor_add(
                        out=yo[:sz], in0=yo[:sz], in1=xs_t[i][:sz]
                    )
                    nc.sync.dma_start(
                        out=y_d[i * P:i * P + sz, :], in_=yo[:sz]
                    )

            # keep-alive so DCE cannot drop the warm-up chain; rides the
            # gpsimd queue at the very end so it never stalls weight DMAs
            nc.scalar.activation(out=wkeep, in_=wps[:, 0:1], func=ACT.Copy)
            nc.gpsimd.dma_start(out=scr_d[:], in_=wkeep)

    nc.compile()
    return nc


def _get_nc(C):
    if C not in _CACHE:
        _CACHE[C] = _build(C)
    return _CACHE[C]


def _route(feats, centroids):
    """Token->expert assignment + gate, computed the same way the reference
    does (jax on CPU) so argmax near-ties resolve identically."""
    try:
        import jax
        import jax.numpy as jnp

        with jax.default_device(jax.devices("cpu")[0]):
            scores = jnp.asarray(feats) @ jnp.asarray(centroids).T
            assign = jnp.argmax(scores, axis=1)
            alpha = jax.nn.sigmoid(
                jnp.take_along_axis(scores, assign[:, None], axis=1)
            )
            return np.asarray(assign), np.asarray(alpha, dtype=np.float32)
    except Exception:
        scores = feats @ centroids.T
        assign = np.argmax(scores, axis=1)
        alpha = 1.0 / (1.0 + np.exp(-scores[np.arange(len(assign)), assign]))
        return assign, alpha[:, None].astype(np.float32)


def prepare(x, centroids, ln_g, ln_b, W1, b1, W2, b2):
    """Shard the full inputs: route tokens to experts, apply LN, build
    per-core input maps. Returns (C, in_maps, idx, orig_shape)."""
    bf16 = ml_dtypes.bfloat16
    x = np.asarray(x)
    orig_shape = x.shape
    feats = np.ascontiguousarray(x.reshape(-1, D), dtype=np.float32)
    centroids = np.asarray(centroids, dtype=np.float32)

    assign, alpha = _route(feats, centroids)

    idx = [np.nonzero(assign == e)[0] for e in range(E)]
    max_count = max(len(ix) for ix in idx)
    C = max(256, -(-max_count // 64) * 64)

    W1 = np.asarray(W1, dtype=np.float32)
    W2 = np.asarray(W2, dtype=np.float32)
    b1 = np.asarray(b1, dtype=np.float32)
    b2 = np.asarray(b2, dtype=np.float32)
    ln_g = np.asarray(ln_g, dtype=np.float32)
    ln_b = np.asarray(ln_b, dtype=np.float32)

    # per-token LN on the host (the device receives normalized, transposed
    # activations)
    mu = feats.mean(axis=1, keepdims=True)
    var = np.square(feats - mu).mean(axis=1)
    rs = (1.0 / np.sqrt(var + LN_EPS))[:, None]
    xnorm = (feats - mu) * rs

    NT = -(-C // P)
    FT = F // P
    KT = D // P
    in_maps = []
    for e in range(E):
        ne = len(idx[e])
        al = np.zeros((NT * P,), dtype=np.float32)
        al[:ne] = alpha[idx[e], 0]
        # residual with alpha*b2 folded in (exact reparameterization)
        xs2 = np.zeros((NT * P, D), dtype=np.float32)
        xs2[:ne] = feats[idx[e]] + al[:ne, None] * b2[e][None, :]
        # normalized activations, bf16, transposed to D-major [P, KT*C]
        xp = np.zeros((NT * P, D), dtype=np.float32)
        xp[:ne] = xnorm[idx[e]]
        xp = xp[:C].astype(bf16)
        xlnT = np.concatenate(
            [xp[:, k * P:(k + 1) * P].T for k in range(KT)]
            + [W2[e][:P, :].astype(bf16)],    # w2 quad-0 tile-0 in the head
            axis=1,
        )
        # fold LN affine into the first FFN layer (exact reparameterization)
        w1_eff = ln_g[e][:, None] * W1[e]
        b1_eff = ln_b[e] @ W1[e] + b1[e]

        meta = np.empty((P, FT + NT), dtype=np.float32)
        meta[:, :FT] = b1_eff.reshape(FT, P).T
        meta[:, FT:] = al.reshape(NT, P).T

        wall = np.empty((2 * (FT // 4), P, KT * 512), dtype=bf16)
        for g in range(FT // 4):
            wall[2 * g] = (
                w1_eff[:, g * 512:(g + 1) * 512]
                .reshape(KT, P, 512).transpose(1, 0, 2).reshape(P, KT * 512)
                .astype(bf16)
            )
            wall[2 * g + 1] = (
                W2[e][4 * g * P:(4 * g + 4) * P, :]
                .reshape(4, P, D).transpose(1, 0, 2).reshape(P, 4 * D)
                .astype(bf16)
            )
        in_maps.append(
            dict(
                xlnT=np.ascontiguousarray(xlnT),
                xs=np.ascontiguousarray(xs2.reshape(NT, P, D)),
                meta=meta,
                wall=wall,
            )
        )
    return C, in_maps, idx, orig_shape


def kernel(x, centroids, ln_g, ln_b, W1, b1, W2, b2):
    from concourse.bass_utils import run_bass_kernel_spmd

    C, in_maps, idx, orig_shape = prepare(
        x, centroids, ln_g, ln_b, W1, b1, W2, b2
    )
    nc = _get_nc(C)
    res = run_bass_kernel_spmd(nc, in_maps, core_ids=list(range(E)))

    T = int(np.prod(orig_shape[:-1]))
    out = np.empty((T, D), dtype=np.float32)
    for e in range(E):
        out[idx[e]] = res.results[e]["y"][: len(idx[e])]
    return out.reshape(orig_shape)
